# revision 1
# baseline (speedup 1.0000x reference)
"""Trainium2 Bass kernel for nn_CrossAttention (packed cross-attention), v2.

Math (verified against the jax reference):
  For each batch b, packed pred rows cross-attend to packed ctx rows:
    Q = Xp_b @ Wq ; [K|V] = Xc_b @ Wkv          (Xp_b, Xc_b: [1024, 512])
    out_b = concat_h( softmax(Q_h K_h^T / 8) V_h ) @ Wproj + bproj
  Softmax needs no max-subtraction: |scores| < ~7, exp is safe in fp32.

Sharding: 8 cores = (2 batches) x (4 head-pairs).  Each core computes two
heads of one batch and the partial output projection for those heads
(row-sharded Wproj); the host sums the 4 partials per batch and adds bproj.

v2 changes over the 60.5us baseline (trace-driven; ~54.3us measured):
  - inputs packed on host so every DMA reads one contiguous HBM block;
    one need-ordered sync queue (wk, xc, wq, xp -- the DMA engines serve
    all queues round-robin at ~300GB/s aggregate, so need-order beats
    parallel queues), tiny wv/wp on the gpsimd SWDGE queue
  - PE warmup + tail keepalive: dummy matmuls on a zeroed scratch tile
    hold the PE clock at 2.4GHz (it drops to 1.2GHz after ~100ns idle
    and takes ~3us of continuous busy to ramp back)
  - wide vones: PV weights are [V_h | 64 ones cols] so PSUM rows 64-127
    get Z replicated -- normalization becomes full-width ACT ln/exp(-x)
    + DVE multiply (no [1,n] ops, no broadcast matmuls)
  - combined otn: head0 rows 0-63, head1 rows 64-127 of one tile, so the
    projection is 8 K=128 matmuls against an unpadded wp (was 16 + pads)
  - per-(head, half) kt/qt tiles and psum-pool entry order chosen so the
    first S matmul inherits no bank-reuse anti-dependency and waits only
    the two evacuations it actually needs (they run on both engines)
  - paired o16 staging: one strided out-DMA per two query tiles
"""

import sys

if "/opt/trn_rl_repo" not in sys.path:
    sys.path.insert(0, "/opt/trn_rl_repo")

import numpy as np

B, T, N, C, H = 2, 8, 256, 512, 8
T_CTX = T // 2
HD = C // H            # 64
SEQ = T_CTX * N        # 1024 packed tokens per batch (q and kv)
NCORE = 8
CT_N = C // 128        # 4 contraction tiles over C
KT_N = SEQ // 128      # 8 key tiles
QT_N = SEQ // 128      # 8 query tiles
SCALE = HD ** -0.5
N_WARM = 7

_PROG = None
SPLIT_WAITS = True  # walrus needs it; CoreSim chokes on it


def _build_program():
    import concourse.bass as bass
    import concourse.tile as tile
    from concourse import mybir

    F32 = mybir.dt.float32
    F16 = mybir.dt.float16

    class TrimTailTileContext(tile.TileContext):
        """Skip the second end-of-kernel all-engine barrier: executions of
        the NEFF are serialized by the runtime, and the semaphore clear is
        still ordered after the first barrier on the gpsimd queue."""

        def _drain_and_barrier(self, tick_clock, wait_clock):
            from concourse.vector_clock import ScopedClock

            drain_inst = self.nc.sync.drain()
            wait_clock.add_sem_waits(
                drain_inst.ins, ScopedClock({None: tick_clock.global_clock}))
            self.nc.all_engine_barrier()
            popped = self.nc._tile_sem_poison_stack.pop()
            assert popped is self._sem_poison
            self.nc.clear_and_free_semaphores(
                list(self.sems.allocated().values()))

    nc = bass.Bass("TRN2", target_bir_lowering=False, debug=False,
                   num_devices=NCORE)

    xcP = nc.dram_tensor("xcP", [2, 128, 2, SEQ], F16,
                         kind="ExternalInput").ap()
    xpP = nc.dram_tensor("xpP", [2, 128, 2, SEQ], F16,
                         kind="ExternalInput").ap()
    wq = nc.dram_tensor("wq", [128, CT_N, 128], F16, kind="ExternalInput").ap()
    wk = nc.dram_tensor("wk", [128, CT_N, 128], F16, kind="ExternalInput").ap()
    wv = nc.dram_tensor("wv", [128, CT_N, 128], F16, kind="ExternalInput").ap()
    wp = nc.dram_tensor("wp", [128, C], F16, kind="ExternalInput").ap()
    out = nc.dram_tensor("out", [SEQ, C], F16, kind="ExternalOutput").ap()

    with TrimTailTileContext(nc) as tc:
        _emit(nc, tc, mybir, xcP, xpP, wq, wk, wv, wp, out)
    if SPLIT_WAITS:
        _split_sync_waits(nc, mybir)
    return nc


def _split_sync_waits(nc, mybir):
    """This container's walrus build has tight per-instruction sync-wait
    limits ("Too many sync wait commands": Matmult holds 1 wait command,
    control-class instructions 2).  Tile freely assigns more.  Rewrite each
    block, moving overflow waits onto same-engine NoOps inserted directly
    before the over-limit instruction (safe: the engine queue executes in
    order, so the waits still complete before the instruction runs)."""
    LIMITS = {}
    DEFAULT = 1
    NOP_W = 1
    n = 0
    for fn in nc.m.functions:
        for bb in fn.blocks:
            insts = bb.instructions
            new = []
            changed = False
            for inst in insts:
                si = inst.sync_info
                waits = list(si.on_wait) if si is not None else []
                limit = LIMITS.get(inst.opcode, DEFAULT)
                if len(waits) > limit:
                    extra = waits[:-limit] if limit else waits
                    keep = waits[-limit:] if limit else []
                    # the end-of-kernel drain carries one wait per logical
                    # processor; spread its nops across engines so they
                    # retire in parallel (the following barrier re-syncs),
                    # instead of ~130ns each serially on the sync sequencer
                    if inst.opcode == "Drain" and len(extra) > 4:
                        engs = [mybir.EngineType.SP, mybir.EngineType.PE,
                                mybir.EngineType.DVE,
                                mybir.EngineType.Activation,
                                mybir.EngineType.Pool]
                    else:
                        engs = [inst.engine]
                    for i in range(0, len(extra), NOP_W):
                        nop = mybir.InstNoOp(
                            name=f"I-waitsplit-{n}", ins=[], outs=[],
                            engine=engs[(i // NOP_W) % len(engs)],
                            sync_info=mybir.SyncInfo(
                                on_wait=extra[i:i + NOP_W], on_update=[]))
                        new.append(nop)
                        n += 1
                    inst.sync_info = mybir.SyncInfo(
                        on_wait=keep, on_update=list(si.on_update))
                    changed = True
                new.append(inst)
            if changed:
                bb.instructions = new


def _emit(nc, tc, mybir, xcP, xpP, wq, wk, wv, wp, out):
    from contextlib import ExitStack

    F32 = mybir.dt.float32
    F16 = mybir.dt.float16
    Exp = mybir.ActivationFunctionType.Exp
    Ln = mybir.ActivationFunctionType.Ln

    with ExitStack() as ctx:
        sb = ctx.enter_context(tc.tile_pool(name="sb", bufs=1))

        # separate tiles per DMA chunk / consumer granularity: Tile tracks
        # dependencies at tile granularity
        warm = sb.tile([128, 512], F16, tag="warm")
        xc_sb = [sb.tile([128, 2, SEQ], F16, tag=f"xc{a}", name=f"xc{a}")
                 for a in range(2)]
        xp_sb = [sb.tile([128, 2, SEQ], F16, tag=f"xp{a}", name=f"xp{a}")
                 for a in range(2)]
        wq_sb = sb.tile([128, CT_N, 128], F16, tag="wq")
        wk_sb = sb.tile([128, CT_N, 128], F16, tag="wk")
        wv_sb = sb.tile([128, CT_N, 128], F16, tag="wv")
        wp_sb = sb.tile([128, C], F16, tag="wp")
        # per-(head, column-half) tiles: finer dependency granularity lets
        # the first S matmul start after just two (parallel-engine) evacs
        qt_p = [[sb.tile([128, 512], F16, tag=f"qt{h}{nh}",
                         name=f"qt{h}{nh}") for nh in range(2)]
                for h in range(2)]
        kt_p = [[sb.tile([128, 512], F16, tag=f"kt{h}{g}",
                         name=f"kt{h}{g}") for g in range(2)]
                for h in range(2)]
        # wide vones: per kt tile, per head: 64 V cols then 64 ones cols.
        # PV with this lhsT gives psum rows 0-63 = O_h^T, rows 64-127 = Z
        # replicated 64x (so normalization needs no broadcast).
        vones = [sb.tile([128, 4, 256], F16, tag=f"vones{g}", name=f"vones{g}")
                 for g in range(2)]
        # otn per query-half: head0 rows 0-63, head1 rows 64-127 (combined
        # so projection contracts both heads in one K=128 matmul)
        otn = [sb.tile([128, 512], F16, tag=f"otn{nh}", name=f"otn{nh}")
               for nh in range(2)]
        rbc = [sb.tile([64, SEQ], F16, tag=f"rbc{h}", name=f"rbc{h}")
               for h in range(2)]
        zln = [sb.tile([64, SEQ], F16, tag=f"zln{h}", name=f"zln{h}")
               for h in range(2)]
        # ping-pong work tiles (few allocations -> few tile releases)
        p_t = [sb.tile([128, SEQ], F16, tag=f"pt{i}", name=f"pt{i}")
               for i in range(4)]
        # paired output staging: one contiguous [128, 2, 512] tile per two
        # query tiles -> one strided out-DMA per pair (fewer 600ns issues)
        o16_t = [sb.tile([128, 2, C], F16, tag=f"o16{i}", name=f"o16{i}")
                 for i in range(4)]

        # ---- PE warmup: ramp the clock during the DMA window ----
        nc.gpsimd.memset(warm[:], 0.0)

        # ---- input DMAs: one strictly-ordered queue in need-order (the
        # DMA engines serve all queues round-robin, so parallel queues
        # just make everything finish at wire-end together — need-order
        # on one queue gives kt its xc stream first); the two tiny tail
        # weights ride the gpsimd SWDGE queue in parallel ----
        nc.sync.dma_start(out=wk_sb[:], in_=wk[:])
        nc.sync.dma_start(out=xc_sb[0][:], in_=xcP[0])
        nc.sync.dma_start(out=xc_sb[1][:], in_=xcP[1])
        nc.sync.dma_start(out=wq_sb[:], in_=wq[:])
        nc.sync.dma_start(out=xp_sb[0][:], in_=xpP[0])
        nc.sync.dma_start(out=xp_sb[1][:], in_=xpP[1])
        nc.gpsimd.dma_start(out=wv_sb[:], in_=wv[:])
        nc.gpsimd.dma_start(out=wp_sb[:], in_=wp[:])

        # ---- constant / zero-pad memsets (overlap the DMA window) ----
        for g in range(2):
            nc.vector.memset(vones[g][:, :, 64:128], 1.0)
            nc.vector.memset(vones[g][:, :, 192:256], 1.0)
        for g in range(2):
            nc.gpsimd.memset(kt_p[0][g][64:128, :], 0.0)
            nc.gpsimd.memset(kt_p[1][g][0:64, :], 0.0)
            nc.gpsimd.memset(qt_p[0][g][64:128, :], 0.0)
            nc.gpsimd.memset(qt_p[1][g][0:64, :], 0.0)

        # ---- KT then V then QT on the PE (matches data-arrival order) ----
        with ExitStack() as qctx:
            qkt_pool = qctx.enter_context(
                tc.tile_pool(name="qkt_ps", bufs=1, space="PSUM"))
            v_pool = qctx.enter_context(
                tc.tile_pool(name="v_ps", bufs=1, space="PSUM"))
            warm_pool = qctx.enter_context(
                tc.tile_pool(name="warm_ps", bufs=1, space="PSUM"))
            # allocation order = psum bank order; banks are recycled into
            # pool B in the same order, so put qt/kt first (pv reuses them,
            # and PV starts late) and warm last (s_t[0] reuses it, and the
            # first S matmul must not wait anyone)
            qt_ps = [qkt_pool.tile([128, 512], F32, tag=f"qtps{nh}",
                                   name=f"qtps{nh}") for nh in range(2)]
            kt_ps = [qkt_pool.tile([128, 512], F32, tag=f"ktps{nh}",
                                   name=f"ktps{nh}") for nh in range(2)]
            v_ps = [v_pool.tile([128, 128], F32, tag=f"vps{i}",
                                name=f"vps{i}") for i in range(2)]
            warm_ps = warm_pool.tile([128, 512], F32, tag="warmps")

            for i in range(N_WARM):
                nc.tensor.matmul(out=warm_ps[:], lhsT=warm[:, 0:128],
                                 rhs=warm[:], start=True, stop=True)

            for ct in range(CT_N):
                a, j = ct // 2, ct % 2
                for nh in range(2):
                    nc.tensor.matmul(
                        out=kt_ps[nh][:],
                        lhsT=wk_sb[:, ct, :],
                        rhs=xc_sb[a][:, j, nh * 512:(nh + 1) * 512],
                        start=(ct == 0), stop=(ct == CT_N - 1))
            # evacuations: one engine per destination tile (cross-engine
            # writes to one tile serialize through an extra semaphore hop);
            # h=0 on vector, h=1 on scalar so both run concurrently
            for nh in range(2):
                nc.vector.tensor_copy(out=kt_p[0][nh][0:64, :],
                                      in_=kt_ps[nh][0:64, :])
                nc.scalar.copy(out=kt_p[1][nh][64:128, :],
                               in_=kt_ps[nh][64:128, :])

            for kt in range(KT_N):
                vt = v_ps[kt % 2]
                for ct in range(CT_N):
                    a, j = ct // 2, ct % 2
                    nc.tensor.matmul(
                        out=vt[:],
                        lhsT=xc_sb[a][:, j, kt * 128:(kt + 1) * 128],
                        rhs=wv_sb[:, ct, :],
                        start=(ct == 0), stop=(ct == CT_N - 1))
                dst = vones[kt // 4][:, kt % 4, :].rearrange(
                    "p (g s) -> p g s", g=2)[:, :, 0:64]
                vsrc = vt[:].rearrange("p (g s) -> p g s", g=2)
                nc.vector.tensor_copy(out=dst, in_=vsrc)

            for ct in range(CT_N):
                a, j = ct // 2, ct % 2
                for nh in range(2):
                    nc.tensor.matmul(
                        out=qt_ps[nh][:],
                        lhsT=wq_sb[:, ct, :],
                        rhs=xp_sb[a][:, j, nh * 512:(nh + 1) * 512],
                        start=(ct == 0), stop=(ct == CT_N - 1))
            # spread the four qt evacs so item 0's pair (h=0, both query
            # halves) runs on two different engines concurrently
            nc.vector.tensor_copy(out=qt_p[0][0][0:64, :],
                                  in_=qt_ps[0][0:64, :])
            nc.scalar.copy(out=qt_p[0][1][0:64, :],
                           in_=qt_ps[1][0:64, :])
            nc.scalar.copy(out=qt_p[1][0][64:128, :],
                           in_=qt_ps[0][64:128, :])
            nc.vector.tensor_copy(out=qt_p[1][1][64:128, :],
                                  in_=qt_ps[1][64:128, :])

        # ---- attention: S^T -> exp -> PV (Z replicated via ones cols) ----
        with ExitStack() as actx:
            pv_pool = actx.enter_context(
                tc.tile_pool(name="pv_ps", bufs=1, space="PSUM"))
            pv = [pv_pool.tile([128, SEQ], F32, tag=f"pv{i}", name=f"pv{i}")
                  for i in range(2)]
            s_stack = ExitStack()
            s_pool = s_stack.enter_context(
                tc.tile_pool(name="s_ps", bufs=1, space="PSUM"))
            # s_t[0] allocated LAST so it lands on the warm/free banks
            # (earliest retired) -- the first S matmul must not inherit an
            # anti-dependency on the qt/v evacuations via bank reuse
            s_t1 = s_pool.tile([128, SEQ], F32, tag="st1", name="st1")
            s_t0 = s_pool.tile([128, SEQ], F32, tag="st0", name="st0")
            s_t = [s_t0, s_t1]
            items = [(kt, h) for kt in range(KT_N) for h in range(2)]

            def emit_st(i):
                kt, h = items[i]
                s = s_t[i % 2]
                q = (kt % 4) * 128
                for nh in range(2):
                    nc.tensor.matmul(
                        out=s[:, nh * 512:(nh + 1) * 512],
                        lhsT=kt_p[h][kt // 4][:, q:q + 128],
                        rhs=qt_p[h][nh][:],
                        start=True, stop=True)

            emit_st(0)
            for i, (kt, h) in enumerate(items):
                if i + 1 < len(items):
                    emit_st(i + 1)
                p = p_t[i % 4]
                nc.scalar.activation(out=p[:], in_=s_t[i % 2][:], func=Exp,
                                     scale=float(SCALE))
                for nh in range(2):
                    nc.tensor.matmul(
                        out=pv[h][:, nh * 512:(nh + 1) * 512],
                        lhsT=vones[kt // 4][:, kt % 4,
                                            h * 128:(h + 1) * 128],
                        rhs=p[:, nh * 512:(nh + 1) * 512],
                        start=(kt == 0), stop=(kt == KT_N - 1))
            s_stack.close()

            # ---- tail, chunked by query half so projection starts early.
            # 1/Z full-width: nh=0 via DVE reciprocal, nh=1 via ACT
            # ln -> exp(-x) (same act table set as Exp), then otn = O^T/Z
            # and the K=128 projection per query tile.
            with ExitStack() as tctx:
                out_pool = tctx.enter_context(
                    tc.tile_pool(name="out_ps", bufs=1, space="PSUM"))
                out_ps = [out_pool.tile([128, C], F32, tag=f"ops{i}",
                                        name=f"ops{i}") for i in range(4)]
                # PE keepalive through the ln/exp window: an idle PE drops
                # to the 1.2GHz pstate within ~100ns and needs 3us of
                # continuous work to get back to 2.4GHz -- dummy matmuls on
                # the scratch tile bridge the gap until the projections
                for i in range(14):
                    nc.tensor.matmul(out=out_ps[2 + i % 2][:],
                                     lhsT=warm[:, 0:128], rhs=warm[:],
                                     start=True, stop=True)
                # 1/Z = exp(-ln Z) on the replicated Z rows, full width per
                # head (ln and Exp share an act table set: no reload).
                # NOTE: DVE alternatives measured/tried and rejected --
                # native reciprocal is 6558ns for 1024 cols (vs 1106ns per
                # ACT op), and reciprocal_approx_fast trips the walrus
                # "ISA wrong length" codegen bug (custom-DVE ops).
                for h in range(2):
                    nc.scalar.activation(out=zln[h][:],
                                         in_=pv[h][64:128, :], func=Ln)
                    nc.scalar.activation(out=rbc[h][:], in_=zln[h][:],
                                         func=Exp, scale=-1.0)
                for nh in range(2):
                    o = nh * 512
                    for h in range(2):
                        nc.vector.tensor_mul(
                            out=otn[nh][h * 64:(h + 1) * 64, :],
                            in0=pv[h][0:64, o:o + 512],
                            in1=rbc[h][:, o:o + 512])
                    for qt in range(nh * 4, nh * 4 + 4):
                        q = (qt % 4) * 128
                        ot = out_ps[qt % 4]
                        nc.tensor.matmul(out=ot[:],
                                         lhsT=otn[nh][:, q:q + 128],
                                         rhs=wp_sb[:], start=True,
                                         stop=True)
                        o16 = o16_t[qt // 2]
                        if qt % 2 == 0:
                            nc.vector.tensor_copy(out=o16[:, 0, :],
                                                  in_=ot[:])
                        else:
                            nc.scalar.copy(out=o16[:, 1, :], in_=ot[:])
                            # alternate out-DMAs across two queues (sync
                            # HWDGE / idle gpsimd SWDGE) so pairs overlap
                            eng = nc.sync if qt % 4 == 1 else nc.gpsimd
                            eng.dma_start(
                                out=out[(qt - 1) * 128:(qt + 1) * 128, :]
                                .rearrange("(k p) c -> p k c", p=128),
                                in_=o16[:])


def _get_program():
    global _PROG
    if _PROG is None:
        _PROG = _build_program()
    return _PROG


def _shard_inputs(x_pred, x_ctx, ctx_mask, Wq, Wkv, Wproj):
    """Build the 8 per-core input maps (host-side sharding + packing)."""
    ctx_mask = np.asarray(ctx_mask).astype(bool)
    pidx = np.nonzero(~ctx_mask.reshape(-1))[0]
    cidx = np.nonzero(ctx_mask.reshape(-1))[0]
    pm = [np.where(pidx // T == b)[0] for b in range(B)]
    cm = [np.where(cidx // T == b)[0] for b in range(B)]
    for b in range(B):
        assert len(pm[b]) == T_CTX and len(cm[b]) == T_CTX, (
            "kernel compiled for T_CTX ctx/pred slots per batch row")

    def pack_x(X):  # [SEQ, C] -> [2, 128, 2, SEQ], chunk-major so each
        # half-DMA reads one contiguous 512KB HBM block (strided HBM reads
        # halve DMA bandwidth); xc_sb[a][:, j, :] = C-chunk ct = 2a+j
        xt = X.T.astype(np.float16)                 # [C, SEQ]
        return np.ascontiguousarray(
            xt.reshape(2, 2, 128, SEQ).transpose(0, 2, 1, 3))

    def pack_w(W):  # [C, 128] -> [128, CT_N, 128]
        return np.ascontiguousarray(
            W.astype(np.float16).reshape(CT_N, 128, 128).transpose(1, 0, 2))

    xpP_b = [pack_x(x_pred[pm[b]].reshape(SEQ, C)) for b in range(B)]
    xcP_b = [pack_x(x_ctx[cm[b]].reshape(SEQ, C)) for b in range(B)]

    wq16 = Wq.astype(np.float16)
    wk16 = Wkv[:, :C].astype(np.float16)
    wv16 = Wkv[:, C:].astype(np.float16)
    wp16 = Wproj.astype(np.float16)

    in_maps = []
    for c in range(NCORE):
        b, hp = divmod(c, 4)
        hc = hp * 128
        in_maps.append({
            "xpP": xpP_b[b],
            "xcP": xcP_b[b],
            "wq": pack_w(wq16[:, hc:hc + 128]),
            "wk": pack_w(wk16[:, hc:hc + 128]),
            "wv": pack_w(wv16[:, hc:hc + 128]),
            "wp": np.ascontiguousarray(wp16[hc:hc + 128, :]),
        })
    return in_maps, pm


def _unshard_output(results, pm, bproj, dtype):
    full = np.zeros((B * T_CTX, N, C), dtype)
    for b in range(B):
        acc = results[4 * b]["out"].astype(np.float64)
        for j in range(1, 4):
            acc = acc + results[4 * b + j]["out"]
        acc = (acc + bproj).astype(dtype)
        full[pm[b]] = acc.reshape(T_CTX, N, C)
    return full


def run(inputs, trace=False, **kwargs):
    """Run the SPMD kernel; returns (full_output, BassKernelResults)."""
    from concourse.bass_utils import run_bass_kernel_spmd

    nc = _get_program()
    in_maps, pm = _shard_inputs(inputs["x_pred"], inputs["x_ctx"],
                                inputs["ctx_mask"], inputs["Wq"],
                                inputs["Wkv"], inputs["Wproj"])
    res = run_bass_kernel_spmd(nc, in_maps, list(range(NCORE)), trace=trace,
                               **kwargs)
    out = _unshard_output(res.results, pm, np.asarray(inputs["bproj"]),
                          np.asarray(inputs["x_pred"]).dtype)
    return out, res


def kernel(x_pred, x_ctx, ctx_mask, Wq, Wkv, Wproj, bproj):
    out, _ = run(dict(x_pred=np.asarray(x_pred), x_ctx=np.asarray(x_ctx),
                      ctx_mask=np.asarray(ctx_mask), Wq=np.asarray(Wq),
                      Wkv=np.asarray(Wkv), Wproj=np.asarray(Wproj),
                      bproj=np.asarray(bproj)))
    return out

